# revision 13
# baseline (speedup 1.0000x reference)
"""Trainium2 Bass kernel for 3-layer GAT + global max pool + MLP (nn_ATTGCN).

v6 (dma_gather with lo/hi split tables, 256-wide bf16 rows): tile-aligned edge groups (128 dst nodes per node-tile, K_t chunks baked
per tile, identical across cores), per-edge e_d computed on-chip (no e_d
gathers), self-loops handled locally (no gather), static scatter of tile
outputs, static pool loads. Only per-edge row gathers remain on gpsimd.

Core c owns dst-nodes [6250c, 6250(c+1)) and graphs [32c, 32(c+1)).
Self-contained: hardcodes N=50000, E=640000, G=256, F=H=128, L=3, C=10.
"""
import numpy as np

N, E, F, G, C, L = 50000, 640000, 128, 256, 10, 3
NCORES = 8
NPC = N // NCORES            # 6250
NT = (NPC + 127) // 128      # 49
NPAD = NT * 128              # 6272
ROWW = 256                   # h+b(128) | e_s | 1 | zero-pad (512B rows)
RWU = 130                    # used row prefix
NEG_SLOPE = 0.2


def _build_tables(edge_index):
    """Per-core gather-index/dst-rel tables for random edges (no self-loops),
    grouped by dst node-tile, each tile split into lo/hi src-table halves
    with common chunk counts K_lo_t/K_hi_t across cores."""
    HROWS = 3200                       # per-core rows in table A
    src_all = edge_index[0].astype(np.int64)
    dst_all = edge_index[1].astype(np.int64)
    owner = src_all // NPC
    src_aug = owner * NPAD + (src_all - owner * NPC)

    per_core = []
    cnt_lo = np.zeros((NCORES, NT), np.int64)
    cnt_hi = np.zeros((NCORES, NT), np.int64)
    for c in range(NCORES):
        m = (dst_all // NPC) == c
        sa = src_aug[m]
        dl = dst_all[m] - c * NPC
        # sort by (tile, is_hi) then stable
        t = dl // 128
        hi = ((sa % NPAD) >= HROWS).astype(np.int64)
        order = np.lexsort((hi, t))
        sa, dl, t, hi = sa[order], dl[order], t[order], hi[order]
        for tt in range(NT):
            cnt_lo[c, tt] = ((t == tt) & (hi == 0)).sum()
            cnt_hi[c, tt] = ((t == tt) & (hi == 1)).sum()
        per_core.append((sa, dl, t, hi))

    K_lo = (-(-cnt_lo.max(axis=0) // 128)).astype(np.int64)
    K_hi = (-(-cnt_hi.max(axis=0) // 128)).astype(np.int64)
    K = K_lo + K_hi
    CB = np.concatenate([[0], np.cumsum(K)])
    NCH = int(CB[-1])

    tabs = []
    for c in range(NCORES):
        sa, dl, t, hi = per_core[c]
        dst_rel = np.full((128, NCH), -1.0, np.float32)
        idx_lin = np.zeros(NCH * 128, np.int64)
        for tt in range(NT):
            for h, Kh in ((0, int(K_lo[tt])), (1, int(K_hi[tt]))):
                if Kh == 0:
                    continue
                m = (t == tt) & (hi == h)
                ne = int(m.sum())
                nslot = Kh * 128
                base_col = int(CB[tt]) + (int(K_lo[tt]) if h else 0)
                sbuf = np.zeros(nslot, np.int64)
                rbuf = np.full(nslot, -1.0, np.float32)
                loc = sa[m] % NPAD
                own = sa[m] // NPAD
                if h:
                    sbuf[:ne] = own * (NPAD - HROWS) + (loc - HROWS)
                else:
                    sbuf[:ne] = own * HROWS + loc
                rbuf[:ne] = (dl[m] - 128 * tt).astype(np.float32)
                dst_rel[:, base_col:base_col + Kh] = \
                    rbuf.reshape(Kh, 128).T
                idx_lin[base_col * 128:(base_col + Kh) * 128] = sbuf
        # idx tile: linear position i = s*16 + p%16 at [p, s], replicated
        # across the eight 16-partition blocks; a call's columns are the
        # contiguous slice [base_col*8, (base_col+K)*8)
        idx_cols = idx_lin.reshape(NCH * 8, 16).astype(np.int16)  # [S, 16]
        idx16 = np.tile(idx_cols.T, (8, 1))                       # [128, S]
        tabs.append(dict(idx16=idx16, dst_rel=dst_rel))
    return K, K_lo, K_hi, CB, NCH, tabs


def _pool_ranges():
    """Graph node ranges relative to the owning core (identical per core)."""
    gb = np.ceil(np.arange(33) * N / G).astype(np.int64)  # 32 graphs/core
    return [(int(gb[i]), int(gb[i + 1])) for i in range(32)]


def _build_bass(K, K_lo, K_hi, CB, NCH, deltas):
    import concourse.bass as bass
    import concourse.bacc as bacc
    import concourse.mybir as mybir
    import concourse.tile as tile
    import contextlib

    f32 = mybir.dt.float32
    bf16 = mybir.dt.bfloat16
    i16 = mybir.dt.int16
    AF = mybir.ActivationFunctionType
    OP = mybir.AluOpType
    AX = mybir.AxisListType
    IOff = bass.IndirectOffsetOnAxis
    KMAX = int(max(K))

    HROWS = 3200
    nc = bacc.Bacc("TRN2", target_bir_lowering=False, debug=False,
                   num_devices=NCORES, num_swdge_queues=4,
                   dynamic_dma_scratch_size=32768)

    xT_d = nc.dram_tensor("xT", [128, NPAD], f32, kind="ExternalInput").ap()
    Wc_d = nc.dram_tensor("Wc", [128, L * 128], f32, kind="ExternalInput").ap()
    asd_d = nc.dram_tensor("asd", [128, L * 2], f32, kind="ExternalInput").ap()
    bc_d = nc.dram_tensor("bc", [128, L], f32, kind="ExternalInput").ap()
    W1_d = nc.dram_tensor("W1", [128, 128], f32, kind="ExternalInput").ap()
    b1_d = nc.dram_tensor("b1", [128, 1], f32, kind="ExternalInput").ap()
    W2_d = nc.dram_tensor("W2", [128, C], f32, kind="ExternalInput").ap()
    b2_d = nc.dram_tensor("b2", [C, 1], f32, kind="ExternalInput").ap()
    iota_d = nc.dram_tensor("iota", [128, 128], f32, kind="ExternalInput").ap()
    ohn_d = nc.dram_tensor("ohntab", [128, NCH * 128], bf16,
                           kind="ExternalInput").ap()
    ident_d = nc.dram_tensor("ident", [128, 128], f32, kind="ExternalInput").ap()

    idx_d = nc.dram_tensor("idx16", [128, NCH * 8], i16, kind="ExternalInput").ap()
    dstr_d = nc.dram_tensor("dst_rel", [128, NCH], f32, kind="ExternalInput").ap()
    ohtab_d = nc.dram_tensor("ohtab", [128, NCH * 128], bf16,
                             kind="ExternalInput").ap()
    y_d = nc.dram_tensor("y", [32, C], f32, kind="ExternalOutput").ap()

    with tile.TileContext(nc) as tc:
        with contextlib.ExitStack() as ctx:
            cpool = ctx.enter_context(tc.tile_pool(name="consts", bufs=1))
            dram = ctx.enter_context(tc.tile_pool(name="dram", bufs=1, space="DRAM"))

            def const(cname, shape, dt, src):
                t = cpool.tile(shape, dt, name=cname, tag=cname)
                nc.sync.dma_start(t[:], src)
                return t


            Wc = const("c_Wc", [128, L * 128], f32, Wc_d[:])
            asd = const("c_asd", [128, L * 2], f32, asd_d[:])
            bcc = const("c_bcc", [128, L], f32, bc_d[:])
            W1 = const("c_W1", [128, 128], f32, W1_d[:])
            b1 = const("c_b1", [128, 1], f32, b1_d[:])
            W2 = const("c_W2", [128, C], f32, W2_d[:])
            b2 = const("c_b2", [C, 1], f32, b2_d[:])
            iota = const("c_iota", [128, 128], f32, iota_d[:])
            ident = const("c_ident", [128, 128], f32, ident_d[:])

            idx16 = const("c_idx16", [128, NCH * 8], i16, idx_d[:])
            dstr = const("c_dstr", [128, NCH], f32, dstr_d[:])

            # per-layer e_d stage (kept in SBUF dense->edge)
            edst = [cpool.tile([128, NT], f32, name=f"edst{l}") for l in range(L)]
            esst = [cpool.tile([128, NT], f32, name=f"esst{l}") for l in range(L)]

            ag_in = [dram.tile([NPAD, ROWW], bf16, name=f"ag_in{l}")
                     for l in range(L)]
            h_augA = [dram.tile([NCORES * HROWS, ROWW], bf16,
                                addr_space="Shared", name=f"h_augA{l}")
                      for l in range(L)]
            h_augB = [dram.tile([NCORES * (NPAD - HROWS), ROWW], bf16,
                                addr_space="Shared", name=f"h_augB{l}")
                      for l in range(L)]
            hn = [cpool.tile([128, NT, 128], f32, name=f"hn{i}")
                  for i in range(2)]

            dpool = ctx.enter_context(tc.tile_pool(name="dense", bufs=3))
            ps_h = ctx.enter_context(tc.tile_pool(name="psh", bufs=2, space="PSUM"))
            ps_t = ctx.enter_context(tc.tile_pool(name="pst", bufs=2, space="PSUM"))
            ps_e = ctx.enter_context(tc.tile_pool(name="pse", bufs=2, space="PSUM"))

            # ---------------- dense phase ----------------
            def dense_tile(l, t):
                    if l == 0:
                        xt_t = dpool.tile([128, 128], f32, tag="xt_t")
                        nc.sync.dma_start(xt_t[:], xT_d[:, t * 128:(t + 1) * 128])
                        rhsT = xt_t[:]
                    else:
                        pT = ps_t.tile([128, 128], f32, tag='pst')
                        nc.tensor.transpose(pT[:], hn[(l - 1) % 2][:, t, :],
                                            ident[:])
                        hTin = dpool.tile([128, 128], f32, tag="hTin")
                        nc.scalar.activation(hTin[:], pT[:], AF.Identity)
                        rhsT = hTin[:]
                    ph = ps_h.tile([128, 128], f32, tag='psh')
                    nc.tensor.matmul(ph[:], lhsT=Wc[:, l * 128:(l + 1) * 128],
                                     rhs=rhsT, start=True, stop=True)
                    hTb = dpool.tile([128, 128], f32, tag="hTb")
                    nc.scalar.activation(hTb[:], ph[:], AF.Identity,
                                         bias=bcc[:, l:l + 1])
                    pe = ps_e.tile([128, 2], f32, tag='pse')
                    nc.tensor.matmul(pe[:], lhsT=hTb[:],
                                     rhs=asd[:, 2 * l:2 * l + 2],
                                     start=True, stop=True)
                    pr = ps_t.tile([128, 128], f32, tag='pst')
                    nc.tensor.transpose(pr[:], hTb[:], ident[:])
                    rows = dpool.tile([128, ROWW], bf16, tag="rows")
                    nc.scalar.activation(rows[:, 0:128], pr[:], AF.Identity)
                    nc.scalar.activation(rows[:, 128:129], pe[:, 0:1],
                                         AF.Identity)
                    nc.vector.memset(rows[:, 129:130], 1.0)
                    nc.vector.memset(rows[:, 130:256], 0.0)
                    nc.scalar.activation(edst[l][:, t:t + 1], pe[:, 1:2],
                                         AF.Identity, bias=-float(deltas[l]))
                    nc.scalar.activation(esst[l][:, t:t + 1], pe[:, 0:1],
                                         AF.Identity)
                    nc.sync.dma_start(
                        ag_in[l][:].rearrange("(t p) f -> t p f", p=128)[t],
                        rows[:])

            def dense(l):
                for t in range(NT):
                    dense_tile(l, t)

            # ---------------- edge phase ----------------
            gpool = ctx.enter_context(tc.tile_pool(name="gath", bufs=4))
            wpool = ctx.enter_context(tc.tile_pool(name="wchain", bufs=3))
            apool = ctx.enter_context(tc.tile_pool(name="amat", bufs=4))
            spool = ctx.enter_context(tc.tile_pool(name="small", bufs=8))
            hlpool = ctx.enter_context(tc.tile_pool(name="hloc", bufs=3))
            ps_a = ctx.enter_context(tc.tile_pool(name="psagg", bufs=2, space="PSUM"))
            opool = ctx.enter_context(tc.tile_pool(name="ohtp", bufs=3))

            def edge_tile(l, t):
                if True:
                    Kt = int(K[t])
                    cb = int(CB[t])
                    # local rows (self-loop): [h+b | e_s | 1] for this tile
                    hl = hlpool.tile([128, ROWW], bf16, tag="hl")
                    nc.sync.dma_start(
                        hl[:], ag_in[l][:].rearrange(
                            "(t p) f -> t p f", p=128)[t])
                    hlf = hlpool.tile([128, 128], f32, tag="hlf")
                    nc.scalar.activation(hlf[:], hl[:, 0:128], AF.Identity)
                    # e_d column for this tile in bf16 (matmul rhs)
                    egvb = spool.tile([128, 1], bf16, tag="egvb")
                    nc.scalar.activation(egvb[:], edst[l][:, t:t + 1],
                                         AF.Identity)
                    # gather rows for all chunks of this tile
                    pa = ps_a.tile([128, RWU + KMAX], f32, tag='psa')
                    gts = gpool.tile([128, KMAX, ROWW], bf16, tag="gts")
                    klo, khi = int(K_lo[t]), int(K_hi[t])
                    if klo > 0:
                        nc.gpsimd.dma_gather(
                            out_ap=gts[:, 0:klo, :],
                            in_ap=h_augA[l][:],
                            idxs_ap=idx16[:, cb * 8:(cb + klo) * 8],
                            num_idxs=klo * 128, num_idxs_reg=klo * 128,
                            elem_size=ROWW, queue_num=(2 * t) % 4)
                    if khi > 0:
                        nc.gpsimd.dma_gather(
                            out_ap=gts[:, klo:klo + khi, :],
                            in_ap=h_augB[l][:],
                            idxs_ap=idx16[:, (cb + klo) * 8:(cb + Kt) * 8],
                            num_idxs=khi * 128, num_idxs_reg=khi * 128,
                            elem_size=ROWW, queue_num=(2 * t + 1) % 4)
                    # per-edge e_d: host-prebaked transposed one-hots,
                    # one 1-col matmul per chunk
                    oht = opool.tile([128, KMAX, 128], bf16, tag="oht")
                    nc.sync.dma_start(
                        oht[:, 0:Kt, :],
                        ohtab_d[:, cb * 128:(cb + Kt) * 128].rearrange(
                            "p (k e) -> p k e", e=128))
                    ohn = opool.tile([128, KMAX, 128], bf16, tag="ohn")
                    nc.sync.dma_start(
                        ohn[:, 0:Kt, :],
                        ohn_d[:, cb * 128:(cb + Kt) * 128].rearrange(
                            "p (k e) -> p k e", e=128))
                    for k in range(Kt):
                        nc.tensor.matmul(pa[:, RWU + k:RWU + k + 1],
                                         lhsT=oht[:, k, :],
                                         rhs=egvb[:], start=True, stop=True)
                    # s = e_s[src] + e_d[dst]; leaky; exp
                    s = wpool.tile([128, KMAX], f32, tag="s")
                    nc.vector.tensor_tensor(
                        out=s[:, 0:Kt], in0=gts[:, 0:Kt, 128],
                        in1=pa[:, RWU:RWU + Kt], op=OP.add)
                    t2 = wpool.tile([128, KMAX], f32, tag="t2")
                    nc.vector.tensor_scalar(
                        out=t2[:, 0:Kt], in0=s[:, 0:Kt],
                        scalar1=NEG_SLOPE, scalar2=None, op0=OP.mult)
                    lk = wpool.tile([128, KMAX], f32, tag="lk")
                    nc.vector.tensor_tensor(
                        out=lk[:, 0:Kt], in0=s[:, 0:Kt],
                        in1=t2[:, 0:Kt], op=OP.max)
                    w = wpool.tile([128, KMAX], f32, tag="w")
                    nc.scalar.activation(w[:, 0:Kt], lk[:, 0:Kt], AF.Exp)

                    # self-loop weight: exp(leaky(e_s_local + e_d_local))
                    ssl = spool.tile([128, 1], f32, tag="ssl")
                    nc.vector.tensor_tensor(
                        out=ssl[:], in0=esst[l][:, t:t + 1],
                        in1=edst[l][:, t:t + 1], op=OP.add)
                    ts2 = spool.tile([128, 1], f32, tag="ts2")
                    nc.vector.tensor_scalar(
                        out=ts2[:], in0=ssl[:], scalar1=NEG_SLOPE,
                        scalar2=None, op0=OP.mult)
                    lsl = spool.tile([128, 1], f32, tag="lsl")
                    nc.vector.tensor_tensor(out=lsl[:], in0=ssl[:],
                                            in1=ts2[:], op=OP.max)
                    wsl = spool.tile([128, 1], f32, tag="wsl")
                    nc.scalar.activation(wsl[:], lsl[:], AF.Exp)
                    # scatter-accumulate via one-hot matmuls
                    for k in range(Kt):
                        rw = apool.tile([128, RWU], bf16, tag=f"rw{k % 4}")
                        nc.scalar.activation(rw[:], gts[:, k, 0:RWU],
                                             AF.Identity,
                                             scale=w[:, k:k + 1])
                        nc.tensor.matmul(pa[:, 0:RWU], lhsT=ohn[:, k, :],
                                         rhs=rw[:],
                                         start=(k == 0), stop=(k == Kt - 1))
                    # normalize with self-loop folded in
                    zb = spool.tile([128, 1], f32, tag="zb")
                    nc.vector.tensor_tensor(out=zb[:], in0=pa[:, 129:130],
                                            in1=wsl[:], op=OP.add)
                    rz = spool.tile([128, 1], f32, tag="rz")
                    nc.vector.reciprocal(rz[:], zb[:])
                    slh = apool.tile([128, 128], f32, tag="slh")
                    nc.scalar.activation(slh[:], hlf[:], AF.Identity,
                                         scale=wsl[:])
                    hsum = apool.tile([128, 128], f32, tag="hsum")
                    nc.vector.tensor_tensor(out=hsum[:], in0=pa[:, 0:128],
                                            in1=slh[:], op=OP.add)
                    nc.scalar.activation(hn[l % 2][:, t, :], hsum[:],
                                         AF.Relu, scale=rz[:])

            # ---------------- run the layers ----------------
            rg = [list(range(NCORES))]
            def ag_half(l, half):
                if half == 0:
                    nc.gpsimd.collective_compute(
                        "AllGather", mybir.AluOpType.bypass,
                        ins=[ag_in[l][0:HROWS, :].opt()],
                        outs=[h_augA[l].opt()], replica_groups=rg)
                else:
                    nc.gpsimd.collective_compute(
                        "AllGather", mybir.AluOpType.bypass,
                        ins=[ag_in[l][HROWS:NPAD, :].opt()],
                        outs=[h_augB[l].opt()], replica_groups=rg)

            dense(0)
            ag_half(0, 0)
            ag_half(0, 1)
            for l in range(L):
                for t in range(NT):
                    edge_tile(l, t)
                    if l + 1 < L:
                        dense_tile(l + 1, t)
                        if t == 24:
                            ag_half(l + 1, 0)
                        elif t == NT - 1:
                            ag_half(l + 1, 1)

            # ---------------- pooling + MLP ----------------
            # transpose each tile of hn[(L-1)%2] once, then per-graph
            # segment max-reduces over column ranges (no DMA)
            pgpool = ctx.enter_context(tc.tile_pool(name="poolg", bufs=1))
            gmax = cpool.tile([128, 32], f32, name="gmax")
            trT = []
            for t in range(NT):
                prr = ps_t.tile([128, 128], f32, tag='pst')
                nc.tensor.transpose(prr[:], hn[(L - 1) % 2][:, t, :], ident[:])
                sb = pgpool.tile([128, 128], f32, tag=f"ptr{t}")
                nc.scalar.activation(sb[:], prr[:], AF.Identity)
                trT.append(sb)
            for gi, (lo, hi) in enumerate(_pool_ranges()):
                segs = []
                t0, t1 = lo // 128, (hi - 1) // 128
                for t in range(t0, t1 + 1):
                    a = max(lo - t * 128, 0)
                    b = min(hi - t * 128, 128)
                    rm = spool.tile([128, 1], f32, tag=f"rm{len(segs)}")
                    nc.vector.tensor_reduce(rm[:], trT[t][:, a:b],
                                            axis=AX.X, op=OP.max)
                    segs.append(rm)
                acc = segs[0]
                for si in range(1, len(segs)):
                    nxt = spool.tile([128, 1], f32, tag=f"rma{si}")
                    nc.vector.tensor_tensor(out=nxt[:], in0=acc[:],
                                            in1=segs[si][:], op=OP.max)
                    acc = nxt
                nc.vector.tensor_copy(gmax[:, gi:gi + 1], acc[:])
            pg1 = ps_h.tile([128, 32], f32, tag='psh')
            nc.tensor.matmul(pg1[:], lhsT=W1[:], rhs=gmax[:], start=True, stop=True)
            g1 = cpool.tile([128, 32], f32, name="g1t")
            nc.scalar.activation(g1[:], pg1[:], AF.Relu, bias=b1[:])
            pl2 = ps_e.tile([C, 32], f32, tag='pse')
            nc.tensor.matmul(pl2[:], lhsT=W2[:], rhs=g1[:], start=True, stop=True)
            lgT = cpool.tile([C, 32], f32, name="lgT")
            nc.scalar.activation(lgT[:], pl2[:], AF.Identity, bias=b2[:])
            plg = ps_t.tile([32, C], f32, tag='pst')
            nc.tensor.transpose(plg[:], lgT[:], ident[:C, :C])
            lg = cpool.tile([32, C], f32, name="lg")
            nc.vector.tensor_copy(lg[:], plg[:])
            mx = cpool.tile([32, 1], f32, name="mx")
            nc.vector.tensor_reduce(mx[:], lg[:], axis=AX.X, op=OP.max)
            tl = cpool.tile([32, C], f32, name="tl")
            nc.vector.tensor_scalar(out=tl[:], in0=lg[:], scalar1=mx[:],
                                    scalar2=None, op0=OP.subtract)
            ex = cpool.tile([32, C], f32, name="ex")
            nc.scalar.activation(ex[:], tl[:], AF.Exp)
            sm = cpool.tile([32, 1], f32, name="sm")
            nc.vector.tensor_reduce(sm[:], ex[:], axis=AX.X, op=OP.add)
            ln = cpool.tile([32, 1], f32, name="ln")
            nc.scalar.activation(ln[:], sm[:], AF.Ln)
            ysb = cpool.tile([32, C], f32, name="ysb")
            nc.vector.tensor_scalar(out=ysb[:], in0=tl[:], scalar1=ln[:],
                                    scalar2=None, op0=OP.subtract)
            nc.sync.dma_start(y_d[:], ysb[:])

    nc.compile()
    return nc


_CACHE = {}


def _get_program(K, K_lo, K_hi, CB, NCH, deltas):
    key = (tuple(int(x) for x in K_lo), tuple(int(x) for x in K_hi), deltas)
    if key not in _CACHE:
        _CACHE[key] = _build_bass(K, K_lo, K_hi, CB, NCH, deltas)
    return _CACHE[key]


import ml_dtypes


def _make_ohntab(dst_rel):
    nch = dst_rel.shape[1]
    # ohn[p, c*128+n] = 1.0 if dst_rel[p, c] == n else 0
    oh3 = (dst_rel.T[:, :, None]
           == np.arange(128, dtype=np.float32)[None, None, :])  # [c, p, n]
    return np.ascontiguousarray(
        oh3.transpose(1, 0, 2).reshape(128, nch * 128)).astype(
        ml_dtypes.bfloat16)


def _make_ohtab(dst_rel):
    nch = dst_rel.shape[1]
    # ohtab[n, c*128+e] = 1.0 if dst_rel[e, c] == n else 0
    oh3 = (np.arange(128, dtype=np.float32)[:, None, None]
           == dst_rel.T[None, :, :])          # [n, c, e]
    return oh3.reshape(128, nch * 128).astype(ml_dtypes.bfloat16)


def run(inputs, trace=False, tmpdir=None):
    from concourse import bass_utils
    x = np.asarray(inputs["x"], np.float32)
    edge_index = np.asarray(inputs["edge_index"], np.int64)
    Wc = np.asarray(inputs["Wc"], np.float32)
    a_src = np.asarray(inputs["a_src"], np.float32)
    a_dst = np.asarray(inputs["a_dst"], np.float32)
    bc = np.asarray(inputs["bc"], np.float32)
    W1 = np.asarray(inputs["W1"], np.float32)
    b1 = np.asarray(inputs["b1"], np.float32)
    W2 = np.asarray(inputs["W2"], np.float32)
    b2 = np.asarray(inputs["b2"], np.float32)

    K, K_lo, K_hi, CB, NCH, tabs = _build_tables(edge_index)
    deltas = tuple(float(bc[l] @ a_src[l] + bc[l] @ a_dst[l]) for l in range(L))
    nc = _get_program(K, K_lo, K_hi, CB, NCH, deltas)

    Wc_in = np.concatenate([Wc[l] for l in range(L)], axis=1)
    asd_in = np.concatenate(
        [np.stack([a_src[l], a_dst[l]], axis=-1) for l in range(L)], axis=1)
    bc_in = np.ascontiguousarray(bc.T)
    iota = np.broadcast_to(np.arange(128, dtype=np.float32), (128, 128)).copy()
    ident = np.eye(128, dtype=np.float32)


    in_maps = []
    for c in range(NCORES):
        xTc = np.zeros((128, NPAD), np.float32)
        xTc[:, :NPC] = x[c * NPC:(c + 1) * NPC].T
        in_maps.append(dict(
            xT=xTc, Wc=Wc_in, asd=asd_in, bc=bc_in, W1=W1,
            b1=b1.reshape(128, 1), W2=W2, b2=b2.reshape(C, 1),
            iota=iota, ident=ident,
            ohntab=_make_ohntab(tabs[c]["dst_rel"]),
            idx16=tabs[c]["idx16"], dst_rel=tabs[c]["dst_rel"],
            ohtab=_make_ohtab(tabs[c]["dst_rel"]),
        ))
    res = bass_utils.run_bass_kernel_spmd(
        nc, in_maps, core_ids=list(range(NCORES)), trace=trace,
        tmpdir=tmpdir)
    out = np.concatenate([res.results[c]["y"] for c in range(NCORES)], axis=0)
    return out, res


def kernel(**inputs) -> np.ndarray:
    out, _ = run(inputs, trace=False)
    return out.astype(np.float32)
